# revision 13
# baseline (speedup 1.0000x reference)
"""Causal self-attention (RoPE, B=4 S=2048 D=2048 H=16) on 8 Trainium2 cores.

Sharding: core c = 2*b + hh  ->  batch b = c // 2, head-half hh = c % 2
(8 heads per core).  Each core computes qkv projection for its heads,
attention, and a partial output projection over its 1024 y-features;
the host sums the two partials of each batch.

v2: fully SBUF-resident (no DRAM spills), fp16 inputs/compute with f32
PSUM accumulation, RoPE fused into PSUM eviction, causal diagonal-block
column trimming, software-pipelined attention inner loop.
"""
import sys

try:
    import concourse.bass as _chk  # noqa: F401
except ImportError:
    for p in ("/opt/trn_rl_repo", "/root/.axon_site/_ro/trn_rl_repo"):
        if p not in sys.path:
            sys.path.insert(0, p)

import math
import numpy as np
import ml_dtypes

import concourse.bass as bass
import concourse.tile as tile
from concourse import mybir
from concourse.bass_utils import run_bass_kernel_spmd

N_CORES = 8
B = 4
D = 2048
H = 16
HD = 128
M = 8                     # heads per core
QF = M * HD               # 1024 q/k/v features per core
KT = D // 128             # 16 contraction tiles
SH = 1024                 # phase-1 seq chunk
ROPE_BASE = 10000.0
F32 = mybir.dt.float32
F16 = mybir.dt.bfloat16
NPF16 = np.float16
EXP = mybir.ActivationFunctionType.Exp
SCALE = 1.0 / math.sqrt(HD)
TRIM = True     # trim dead query columns of causal diagonal blocks
SKIPGC = True   # skip_group_check on partial-column psum accumulation


def split_ctrl_waits(nc, maxw=1):
    """Walrus in this env can't encode >1 sem-wait on many instruction
    formats; move extras onto preceding same-engine NoOps."""
    nid = [0]
    for f in nc.m.functions:
        for b in f.blocks:
            new_insts = []
            for inst in b.instructions:
                si = inst.sync_info
                if si is not None and si.on_wait is not None and len(si.on_wait) > maxw:
                    waits = list(si.on_wait)
                    while len(waits) > maxw:
                        chunk, waits = waits[:maxw], waits[maxw:]
                        nid[0] += 1
                        nop = mybir.InstNoOp(
                            name=f"I-waitsplit-{nid[0]}",
                            ins=[], outs=[],
                            sync_info=mybir.SyncInfo(on_wait=chunk, on_update=[]),
                        )
                        nop.engine = inst.engine
                        new_insts.append(nop)
                    si.on_wait = waits
                new_insts.append(inst)
            b.instructions[:] = new_insts


def build_nc(S=2048, repeat=1, stages="ABC", split_waits=True):
    """One SPMD program; all 8 cores run it on different data."""
    nc = bass.Bass("TRN2", debug=False, num_devices=N_CORES)

    xT = nc.dram_tensor("xT", [D, S], F16, kind="ExternalInput")
    wq = nc.dram_tensor("wq", [M, 128, KT * 128], F16, kind="ExternalInput")
    wk = nc.dram_tensor("wk", [M, 128, KT * 128], F16, kind="ExternalInput")
    wv = nc.dram_tensor("wv", [2, 128, KT * 512], F16, kind="ExternalInput")
    wo = nc.dram_tensor("wo", [128, M * 4 * 512], F16, kind="ExternalInput")
    cosf = nc.dram_tensor("cosf", [128, S], F16, kind="ExternalInput")
    sinf = nc.dram_tensor("sinf", [128, S], F16, kind="ExternalInput")  # pre-swapped+sign-folded
    dmasks = nc.dram_tensor("dmasks", [128, 4 * 512], F16, kind="ExternalInput")
    out = nc.dram_tensor("out", [S, D], F32, kind="ExternalOutput")

    with tile.TileContext(nc) as tc:
        with tc.tile_pool(name="const", bufs=1) as constp:
            cos_t = constp.tile([128, S], F16, name="cos_t")
            sin_t = constp.tile([128, S], F16, name="sin_t")
            mask_t = constp.tile([128, 4 * 512], F16, name="mask_t")
            ones_t = constp.tile([128, 128], F16, name="ones_t")
            nc.sync.dma_start(cos_t[:], cosf[:])
            nc.sync.dma_start(sin_t[:], sinf[:])
            nc.sync.dma_start(mask_t[:], dmasks[:])
            nc.vector.memset(ones_t[:], 1.0)

            qr = constp.tile([128, M * S], F16, name="qr")
            kr = constp.tile([128, M * S], F16, name="kr")
            vsb = constp.tile([128, (S // 128) * QF], F16, name="vsb")
            # vsb col = st*1024 + f   (f = m*128 + hd)

            for _rep in range(repeat):
                _body(nc, tc, S, xT, wq, wk, wv, wo, out,
                      cos_t, sin_t, mask_t, ones_t, qr, kr, vsb, stages)

    if split_waits:
        split_ctrl_waits(nc)
    return nc


def _phase1(nc, tc, S, xT, wq, wk, wv, cos_t, sin_t, qr, kr, vsb):
    """QKV projection + fused RoPE, fp16 in, f32 psum, fp16 out."""
    NH = S // SH              # seq halves
    NCX = SH // 512           # 512-chunks per half

    with tc.tile_pool(name="p1", bufs=2) as xp, \
         tc.tile_pool(name="p1wv", bufs=1) as wvp, \
         tc.tile_pool(name="p1w", bufs=2) as wp, \
         tc.tile_pool(name="p1u", bufs=3) as up, \
         tc.tile_pool(name="p1ps", bufs=2, space="PSUM") as psp:

        for h in range(NH):
            xt = xp.tile([128, KT * SH], F16, name=f"xt{h}", tag="xt")
            for k in range(KT):
                nc.sync.dma_start(
                    xt[:, k * SH:(k + 1) * SH],
                    xT[k * 128:(k + 1) * 128, h * SH:(h + 1) * SH])

            # ---- q^T, k^T with fused RoPE (first: small weight loads) ----
            for w_dram, dst, tg in ((wq, qr, "q"), (wk, kr, "k")):
                for m in range(M):
                    wt = wp.tile([128, KT * 128], F16, name=f"w{tg}{h}_{m}", tag="wqk")
                    nc.sync.dma_start(wt[:], w_dram[m])
                    pos = [psp.tile([128, 512], F32, name=f"qp{tg}{h}_{m}_{c}", tag=f"qps{c}")
                           for c in range(NCX)]
                    for k in range(KT):
                        for c in range(NCX):
                            nc.tensor.matmul(
                                pos[c][:],
                                wt[:, k * 128:(k + 1) * 128],
                                xt[:, k * SH + c * 512: k * SH + c * 512 + 512],
                                start=(k == 0), stop=(k == KT - 1))
                    for c in range(NCX):
                        s0 = h * SH + c * 512     # seq offset
                        dslot = dst[:, m * S + s0: m * S + s0 + 512]
                        u = up.tile([128, 512], F16, name=f"u{tg}{h}_{m}_{c}", tag="u")
                        us = up.tile([128, 512], F16, name=f"us{tg}{h}_{m}_{c}", tag="us")
                        nc.vector.tensor_mul(u[:], pos[c][:], sin_t[:, s0:s0 + 512])
                        nc.vector.tensor_mul(dslot, pos[c][:], cos_t[:, s0:s0 + 512])
                        nc.vector.tensor_copy(us[0:64, :], u[64:128, :])
                        nc.vector.tensor_copy(us[64:128, :], u[0:64, :])
                        nc.vector.tensor_add(dslot, dslot, us[:])

            # ---- v: natural [seq, feat] ----
            for ncx in range(2):
                wvt = wvp.tile([128, KT * 512], F16, name=f"wv{h}_{ncx}", tag="wv")
                nc.sync.dma_start(wvt[:], wv[ncx])
                for st in range(SH // 128):
                    ps = psp.tile([128, 512], F32, name=f"vp{h}_{ncx}_{st}", tag="vps")
                    for k in range(KT):
                        nc.tensor.matmul(
                            ps[:],
                            xt[:, k * SH + st * 128: k * SH + st * 128 + 128],
                            wvt[:, k * 512:(k + 1) * 512],
                            start=(k == 0), stop=(k == KT - 1))
                    stg = h * (SH // 128) + st
                    nc.scalar.copy(vsb[:, stg * QF + ncx * 512: stg * QF + ncx * 512 + 512], ps[:])


def _phase23(nc, tc, S, wo, out, mask_t, ones_t, qr, kr, vsb, stages):
    """Attention (qg-major, heads inner) with output-projection matmuls
    interleaved one unit per head-slot: proj is pure-PE work that fills
    the gaps while Activation chews softmax exps."""
    NQ = S // 512
    with tc.tile_pool(name="p2", bufs=1) as cp, \
         tc.tile_pool(name="p2pt", bufs=4) as ptp, \
         tc.tile_pool(name="p2n", bufs=2) as np_, \
         tc.tile_pool(name="p2o", bufs=4) as cop, \
         tc.tile_pool(name="p2ps", bufs=2, space="PSUM") as sps_p, \
         tc.tile_pool(name="p2ac", bufs=2, space="PSUM") as acc_p, \
         tc.tile_pool(name="p3ps", bufs=1, space="PSUM") as cps:

        yhat = cp.tile([128, M * S], F16, name="yhat")
        wo_sb = cp.tile([128, M * 4 * 512], F16, name="wo_sb")
        nc.sync.dma_start(wo_sb[:], wo[:])

        def attention(m, qg):
            qh0 = m * S
            nkt = 4 * qg + 4
            yps = acc_p.tile([128, 512], F32, name=f"yps{m}_{qg}", tag="yps")
            dps = acc_p.tile([128, 512], F32, name=f"dps{m}_{qg}", tag="dps")
            LOOKAHEAD = 2
            pts = {}

            def emit_front(kt):
                j = kt - 4 * qg
                c0 = 128 * j if (j >= 0 and TRIM) else 0
                sp = sps_p.tile([128, 512], F32, name=f"sp{m}_{qg}_{kt}", tag="sps")
                nc.tensor.matmul(
                    sp[:, c0:512],
                    kr[:, qh0 + kt * 128: qh0 + kt * 128 + 128],
                    qr[:, qh0 + qg * 512 + c0: qh0 + qg * 512 + 512],
                    start=True, stop=True)
                pt = ptp.tile([128, 512], F16, name=f"pt{m}_{qg}_{kt}", tag="pt")
                nc.scalar.activation(pt[:, c0:512], sp[:, c0:512], EXP, scale=SCALE)
                if j >= 0:
                    nc.vector.tensor_mul(
                        pt[:, c0:512], pt[:, c0:512],
                        mask_t[:, j * 512 + c0: (j + 1) * 512])
                pts[kt] = (pt, c0)

            def emit_back(kt):
                pt, c0 = pts.pop(kt)
                nc.tensor.matmul(
                    yps[:, c0:512],
                    vsb[:, kt * QF + m * 128: kt * QF + m * 128 + 128],
                    pt[:, c0:512],
                    start=(kt == 0), stop=(kt == nkt - 1),
                    skip_group_check=SKIPGC)
                nc.tensor.matmul(
                    dps[:, c0:512], ones_t[:], pt[:, c0:512],
                    start=(kt == 0), stop=(kt == nkt - 1),
                    skip_group_check=SKIPGC)

            for kt in range(nkt):
                emit_front(kt)
                if kt >= LOOKAHEAD:
                    emit_back(kt - LOOKAHEAD)
            for kt in range(max(0, nkt - LOOKAHEAD), nkt):
                emit_back(kt)

            rec = np_.tile([128, 512], F32, name=f"rec{m}_{qg}", tag="rec")
            nc.vector.reciprocal(rec[:], dps[:])
            nc.vector.tensor_mul(
                yhat[:, qh0 + qg * 512: qh0 + qg * 512 + 512], yps[:], rec[:])

        def proj_unit(st, ocp):
            # two oc chains (oc = 2*ocp, 2*ocp+1) accumulated over heads
            pos = [cps.tile([128, 512], F32, name=f"cpo{st}_{ocp}_{i}", tag=f"cpo{i}")
                   for i in range(2)]
            for m in range(M):
                for i in range(2):
                    oc = 2 * ocp + i
                    nc.tensor.matmul(
                        pos[i][:],
                        yhat[:, m * S + st * 128: m * S + st * 128 + 128],
                        wo_sb[:, (m * 4 + oc) * 512:(m * 4 + oc + 1) * 512],
                        start=(m == 0), stop=(m == M - 1))
            for i in range(2):
                oc = 2 * ocp + i
                ot = cop.tile([128, 512], F32, name=f"cot{st}_{oc}", tag="cot")
                nc.vector.tensor_copy(ot[:], pos[i][:])
                nc.sync.dma_start(
                    out[st * 128:(st + 1) * 128, oc * 512:(oc + 1) * 512], ot[:])

        for qg in range(NQ):
            for m in range(M):
                attention(m, qg)
                if qg >= 1:
                    st = 4 * (qg - 1) + m // 2
                    proj_unit(st, m % 2)
        if stages == "AB":
            with tc.tile_pool(name="dbg2", bufs=2) as dbg:
                for r0 in range(2):
                    t = dbg.tile([128, S], F32, name=f"dbgy{r0}", tag="dbg")
                    nc.vector.tensor_copy(t[:], yhat[:, r0 * S:(r0 + 1) * S])
                    nc.sync.dma_start(out[r0 * 128:(r0 + 1) * 128, 0:S], t[:])
            return
        g = NQ - 1
        for m in range(M):
            proj_unit(4 * g + m // 2, m % 2)


def _body(nc, tc, S, xT, wq, wk, wv, wo, out,
          cos_t, sin_t, mask_t, ones_t, qr, kr, vsb, stages="ABC"):
    _phase1(nc, tc, S, xT, wq, wk, wv, cos_t, sin_t, qr, kr, vsb)

    if stages == "A":
        with tc.tile_pool(name="dbg", bufs=2) as dbg:
            for (src, r0) in ((qr, 0), (kr, 1)):
                t = dbg.tile([128, S], F32, name=f"dbgq{r0}", tag="dbg")
                nc.vector.tensor_copy(t[:], src[:, 0:S])
                nc.sync.dma_start(out[r0 * 128:(r0 + 1) * 128, 0:S], t[:])
            t = dbg.tile([128, QF], F32, name="dbgv", tag="dbgv")
            nc.vector.tensor_copy(t[:], vsb[:, 0:QF])
            nc.sync.dma_start(out[2 * 128:3 * 128, 0:QF], t[:])
        return

    _phase23(nc, tc, S, wo, out, mask_t, ones_t, qr, kr, vsb, stages)


def prep_in_maps(x, positions, Wqkv, Wout, S=2048):
    """Host-side shard/format. Returns per-core input dicts."""
    f16 = ml_dtypes.bfloat16

    # RoPE tables from positions (deinterleaved pair layout)
    inv_freq = 1.0 / (ROPE_BASE ** (np.arange(0, HD, 2, dtype=np.float64) / HD))  # [64]
    pos = np.asarray(positions).astype(np.float64)[:S]
    freq = pos[None, :] * inv_freq[:, None]          # [64, S]
    c = np.cos(freq).astype(np.float32)
    s = np.sin(freq).astype(np.float32)
    cosf = np.vstack([c, c]).astype(f16)              # [128, S]
    # swapped+sign-folded sin: out = P*cos + swap(P*sinf_sw)
    # rows 0:64 -> +s (will be added into odd rows), rows 64:128 -> -s
    sinf = np.vstack([s, -s]).astype(f16)             # [128, S]

    # diagonal causal masks M_j [128, 4*512]: key r (partition), query col c;
    # block j: cols [0,128j) dead, [128j,128j+128) triu (r<=c-128j), rest live
    dm = np.zeros((128, 4, 512), np.float32)
    for j in range(4):
        dm[:, j, 128 * j:128 * (j + 1)] = np.triu(np.ones((128, 128), np.float32))
        dm[:, j, 128 * (j + 1):] = 1.0
    dmasks = dm.reshape(128, 4 * 512).astype(f16)

    # per-head even/odd column permutation for q,k weights
    perm = np.concatenate([np.arange(0, HD, 2), np.arange(1, HD, 2)])

    in_maps = []
    for c_id in range(N_CORES):
        b, hh = c_id // 2, c_id % 2
        xTb = np.ascontiguousarray(x[b, :S, :].T).astype(f16)     # [D, S]
        f0 = hh * QF
        Wq = Wqkv[:, f0:f0 + QF]
        Wk = Wqkv[:, D + f0:D + f0 + QF]
        Wv = Wqkv[:, 2 * D + f0:2 * D + f0 + QF]
        # permute within each head for q, k
        Wqp = Wq.reshape(D, M, HD)[:, :, perm]        # [D, M, 128]
        Wkp = Wk.reshape(D, M, HD)[:, :, perm]
        # wq[m, p, k*128+f] = Wq[k*128+p, m*128+f]
        wq_p = np.ascontiguousarray(
            Wqp.reshape(KT, 128, M, HD).transpose(2, 1, 0, 3).reshape(M, 128, KT * 128)
        ).astype(f16)
        wk_p = np.ascontiguousarray(
            Wkp.reshape(KT, 128, M, HD).transpose(2, 1, 0, 3).reshape(M, 128, KT * 128)
        ).astype(f16)
        # wv[ncx, p, k*512+f] = Wv[k*128+p, ncx*512+f]
        wv_p = np.ascontiguousarray(
            Wv.reshape(KT, 128, 2, 512).transpose(2, 1, 0, 3).reshape(2, 128, KT * 512)
        ).astype(f16)
        # wo[p, (m*4+oc)*512+f] = Wout[f0 + m*128 + p, oc*512 + f]
        Woh = Wout[f0:f0 + QF, :]                     # [1024, 2048]
        wo_p = np.ascontiguousarray(
            Woh.reshape(M, 128, 4, 512).transpose(1, 0, 2, 3).reshape(128, M * 4 * 512)
        ).astype(f16)
        in_maps.append({
            "xT": xTb, "wq": wq_p, "wk": wk_p, "wv": wv_p, "wo": wo_p,
            "cosf": cosf, "sinf": sinf, "dmasks": dmasks,
        })
    return in_maps


def kernel(x, positions, mask, Wqkv, Wout):
    x = np.asarray(x, dtype=np.float32)
    Wqkv = np.asarray(Wqkv, dtype=np.float32)
    Wout = np.asarray(Wout, dtype=np.float32)
    S = x.shape[1]
    nc = build_nc(S=S)
    in_maps = prep_in_maps(x, positions, Wqkv, Wout, S=S)
    res = run_bass_kernel_spmd(nc, in_maps, core_ids=list(range(N_CORES)))
    outs = [res.results[c]["out"] for c in range(N_CORES)]
    full = np.stack([outs[2 * b] + outs[2 * b + 1] for b in range(B)], axis=0)
    return full.astype(np.float32)
